# revision 19
# baseline (speedup 1.0000x reference)
"""Grok1-style MoE (T=8192, D=2048, F=4096, E=8, top_k=2) on 8 trn2 NeuronCores.

Expert-parallel: one expert per core. The router (tiny fp32 GEMM, 0.03% of
FLOPs) runs on host to decide the token->expert sharding; each core runs the
dense fused FFN  scale * (gelu_tanh(x@wg) * (x@wu)) @ wd  over the ~2048
tokens routed to its expert (bf16 matmuls, fp32 PSUM accumulate); host
scatter-adds the two expert contributions per token.

Device layout (everything transposed so no on-device transposes are needed):
  xt  [D, C]   bf16   gathered tokens, transposed
  wg  [32, 128, 2048] bf16  = w_gate re-tiled so wg[f] is an SBUF tile
                              [k-part, f*128+j] per k-chunk (lhsT layout)
  wu  same as wg
  wd  [16, 128, 4096] bf16  = w_down re-tiled likewise (lhsT layout)
  sc  [128, C] f32    combine weight per token, broadcast over partitions
  yt  [D, C]   f32    output, transposed

Per core:  H^T[f,c] = sum_k wg[k,f] * xt[k,c]   (PSUM, 16 accumulating mms)
           P^T[f,c] = gelu_tanh(H^T) * U^T      (ACT + DVE, bf16 to SBUF)
           Y^T[d,c] = sum_f wd[f,d] * P^T[f,c]  (PSUM, 32 accumulating mms)
           out = Y^T * sc                        (DVE, fp32)
C (token capacity per expert) is processed in 2 column-groups so P^T stays
SBUF-resident (no DRAM spill); weights are re-read once per group.
"""
import os
import sys

sys.path.insert(0, "/opt/trn_rl_repo")

import ml_dtypes
import numpy as np

D = 2048
F = 4096
E = 8
KD = D // 128   # 16 k-chunks for gate/up contraction
FT = F // 128   # 32 f-tiles
DT = D // 128   # 16 d-tiles
SOFTCAP = 30.0

_compiled_cache = {}


def _col_subtiles(cg, max_n=512):
    """Split a column-group of width cg into matmul free-dim subtiles.

    bf16 moving operand supports N up to 1024 (PSUM tile spans 2 banks)."""
    subs = []
    off = 0
    while off < cg:
        s = min(max_n, cg - off)
        subs.append((off, s))
        off += s
    return subs


def _build(C):
    import concourse.bass as bass
    import concourse.tile as tile
    from concourse import bacc, mybir

    f32 = mybir.dt.float32
    bf16 = mybir.dt.bfloat16
    GELU = mybir.ActivationFunctionType.Gelu_apprx_tanh

    nc = bacc.Bacc("TRN2", target_bir_lowering=False, debug=False, num_devices=E, num_swdge_queues=4)
    xt = nc.dram_tensor("xt", [D, C], bf16, kind="ExternalInput").ap()
    wg = nc.dram_tensor("wg", [FT, 128, D], bf16, kind="ExternalInput").ap()
    wu = nc.dram_tensor("wu", [FT, 128, D], bf16, kind="ExternalInput").ap()
    wd = nc.dram_tensor("wd", [DT, 128, F], bf16, kind="ExternalInput").ap()
    sc = nc.dram_tensor("sc", [128, C], f32, kind="ExternalInput").ap()
    yt = nc.dram_tensor("yt", [D, C], f32, kind="ExternalOutput").ap()

    # two column-groups: first an exact multiple of 1024 (pure N=512 matmul
    # subtiles), remainder in the second
    g1 = min((C // 1024) * 1024, 1024)
    groups = [(0, g1), (g1, C - g1)] if 0 < g1 < C else [(0, C)]

    with tile.TileContext(nc) as tc:
        with (
            tc.tile_pool(name="xts", bufs=1) as xpool,
            tc.tile_pool(name="pt", bufs=1) as ptpool,
            tc.tile_pool(name="wgp", bufs=5) as wgpool,
            tc.tile_pool(name="wup", bufs=5) as wupool,
            tc.tile_pool(name="wdp", bufs=3) as wdpool,
            tc.tile_pool(name="gel", bufs=2) as gpool,
            tc.tile_pool(name="scp", bufs=1) as spool,
            tc.tile_pool(name="outp", bufs=4) as opool,
            tc.tile_pool(name="ps", bufs=8, space="PSUM") as psum,
        ):
            for g0, cg in groups:
                subs = _col_subtiles(cg)
                # two half-tiles so the first k-chunk matmuls wait on only
                # half the token DMA; alternate the two HW-DGE queues
                xh = KD // 2
                xts_a = xpool.tile([128, xh, cg], bf16, name="xtsa", tag="xtsa")
                xts_b = xpool.tile([128, KD - xh, cg], bf16, name="xtsb", tag="xtsb")
                for k in range(KD):
                    dst = xts_a[:, k, :] if k < xh else xts_b[:, k - xh, :]
                    eng = nc.sync if k % 2 == 0 else nc.scalar
                    eng.dma_start(dst, xt[k * 128 : (k + 1) * 128, g0 : g0 + cg])
                pt = ptpool.tile([128, FT, cg], bf16, name="pt")

                for f in range(FT):
                    wgt = wgpool.tile([128, D], bf16, name="wgt")
                    nc.gpsimd.dma_start(wgt[:], wg[f])
                    wut = wupool.tile([128, D], bf16, name="wut")
                    nc.gpsimd.dma_start(wut[:], wu[f])

                    psH = [psum.tile([128, 512], f32, name="ps", tag="ps") for _ in subs]
                    for k in range(KD):
                        lhs = wgt[:, k * 128 : (k + 1) * 128]
                        for ci, (off, s) in enumerate(subs):
                            nc.tensor.matmul(
                                psH[ci][:, :s],
                                lhs,
                                (xts_a[:, k, off : off + s] if k < xh else xts_b[:, k - xh, off : off + s]),
                                start=(k == 0),
                                stop=(k == KD - 1),
                            )
                    gel = gpool.tile([128, cg], f32, name="gel")
                    for ci, (off, s) in enumerate(subs):
                        nc.scalar.activation(gel[:, off : off + s], psH[ci][:, :s], GELU)

                    psU = [psum.tile([128, 512], f32, name="ps", tag="ps") for _ in subs]
                    for k in range(KD):
                        lhs = wut[:, k * 128 : (k + 1) * 128]
                        for ci, (off, s) in enumerate(subs):
                            nc.tensor.matmul(
                                psU[ci][:, :s],
                                lhs,
                                (xts_a[:, k, off : off + s] if k < xh else xts_b[:, k - xh, off : off + s]),
                                start=(k == 0),
                                stop=(k == KD - 1),
                            )
                    for ci, (off, s) in enumerate(subs):
                        nc.vector.tensor_mul(
                            pt[:, f, off : off + s],
                            gel[:, off : off + s],
                            psU[ci][:, :s],
                        )

                sct = spool.tile([128, cg], f32, name="sct")
                nc.gpsimd.dma_start(sct[:], sc[:, g0 : g0 + cg])
                for d in range(DT):
                    wdt = wdpool.tile([128, F], bf16, name="wdt")
                    nc.gpsimd.dma_start(wdt[:], wd[d])
                    psY = [psum.tile([128, 512], f32, name="ps", tag="ps") for _ in subs]
                    for f in range(FT):
                        lhs = wdt[:, f * 128 : (f + 1) * 128]
                        for ci, (off, s) in enumerate(subs):
                            nc.tensor.matmul(
                                psY[ci][:, :s],
                                lhs,
                                pt[:, f, off : off + s],
                                start=(f == 0),
                                stop=(f == FT - 1),
                            )
                    for ci, (off, s) in enumerate(subs):
                        outt = opool.tile([128, 512], f32, name="outt", tag="outt")
                        nc.vector.tensor_mul(
                            outt[:, :s], psY[ci][:, :s], sct[:, off : off + s]
                        )
                        nc.sync.dma_start(
                            yt[d * 128 : (d + 1) * 128, g0 + off : g0 + off + s],
                            outt[:, :s],
                        )

    nc.compile()
    return nc


def _enable_ntff_tracing():
    """Register the axon NTFF profile hook (the image lacks antenv.axon_hooks,
    so trn_boot's registration silently degraded). Also stub the S3 artifact
    upload, which has no credentials in this container."""
    import types

    try:
        from antenv import axon_hooks  # noqa: F401
    except ImportError:
        import antenv

        mod = types.ModuleType("antenv.axon_hooks")
        holder = [None]
        mod.set_axon_ntff_profile_hook = lambda h: holder.__setitem__(0, h)
        mod.get_axon_ntff_profile_hook = lambda: holder[0]
        sys.modules["antenv.axon_hooks"] = mod
        antenv.axon_hooks = mod
        from trn_agent_boot.trn_boot import _ntff_profile_via_ctypes

        hook = _ntff_profile_via_ctypes("/opt/axon/libaxon_pjrt.so")
        if hook is not None:
            mod.set_axon_ntff_profile_hook(hook)
    from concourse import bass_utils as bu

    bu.upload_artifacts = lambda tmpdir: ""


def kernel(hidden_states, gate_w, w_gate, w_up, w_down, top_k):
    from concourse.bass_utils import run_bass_kernel_spmd

    x = np.ascontiguousarray(np.asarray(hidden_states, dtype=np.float32))
    gw = np.asarray(gate_w, dtype=np.float32)
    k = int(top_k)
    T = x.shape[0]

    # --- host router (matches reference: fp32 gate, tanh softcap, softmax) ---
    logits = (x @ gw).astype(np.float32)
    logits = np.tanh(logits / SOFTCAP) * SOFTCAP
    m = logits.max(axis=1, keepdims=True)
    ex = np.exp(logits - m)
    probs = (ex / ex.sum(axis=1, keepdims=True)).astype(np.float32)
    order = np.argsort(-probs, axis=1, kind="stable")[:, :k]

    tok_ids = []
    counts = np.zeros(E, np.int64)
    sel = np.zeros((T, E), bool)
    for j in range(k):
        sel[np.arange(T), order[:, j]] = True
    for e in range(E):
        ids = np.nonzero(sel[:, e])[0]
        tok_ids.append(ids)
        counts[e] = len(ids)
    C = max(256, int(np.ceil(counts.max() / 4) * 4))

    nc = _compiled_cache.get(C)
    if nc is None:
        nc = _build(C)
        _compiled_cache[C] = nc

    bf = ml_dtypes.bfloat16
    in_maps = []
    for e in range(E):
        ids = tok_ids[e]
        n = len(ids)
        xt = np.zeros((D, C), bf)
        xt[:, :n] = x[ids].T.astype(bf)
        s = np.zeros((C,), np.float32)
        s[:n] = probs[ids, e]
        sc = np.broadcast_to(s[None, :], (128, C)).copy()
        wg_r = np.ascontiguousarray(
            np.asarray(w_gate[e], np.float32)
            .reshape(KD, 128, FT, 128)
            .transpose(2, 1, 0, 3)
            .reshape(FT, 128, D)
        ).astype(bf)
        wu_r = np.ascontiguousarray(
            np.asarray(w_up[e], np.float32)
            .reshape(KD, 128, FT, 128)
            .transpose(2, 1, 0, 3)
            .reshape(FT, 128, D)
        ).astype(bf)
        wd_r = np.ascontiguousarray(
            np.asarray(w_down[e], np.float32)
            .reshape(FT, 128, DT, 128)
            .transpose(2, 1, 0, 3)
            .reshape(DT, 128, F)
        ).astype(bf)
        in_maps.append({"xt": xt, "wg": wg_r, "wu": wu_r, "wd": wd_r, "sc": sc})

    trace = bool(int(os.environ.get("MOE_TRACE", "0")))
    if trace:
        try:
            _enable_ntff_tracing()
        except Exception as exc:  # tracing is best-effort, never block results
            print(f"ntff tracing unavailable: {exc!r}")
            trace = False
    res = run_bass_kernel_spmd(nc, in_maps, list(range(E)), trace=trace)
    if trace:
        kernel.last_exec_time_ns = res.exec_time_ns
        kernel.last_trace = res.instructions_and_trace

    out = np.zeros((T, D), np.float32)
    for e in range(E):
        ids = tok_ids[e]
        n = len(ids)
        if n:
            out[ids] += res.results[e]["yt"][:, :n].T
    return out


# revision 20
# speedup vs baseline: 1.0014x; 1.0014x over previous
"""Grok1-style MoE (T=8192, D=2048, F=4096, E=8, top_k=2) on 8 trn2 NeuronCores.

Expert-parallel: one expert per core. The router (tiny fp32 GEMM, 0.03% of
FLOPs) runs on host to decide the token->expert sharding; each core runs the
dense fused FFN  scale * (gelu_tanh(x@wg) * (x@wu)) @ wd  over the ~2048
tokens routed to its expert (bf16 matmuls, fp32 PSUM accumulate); host
scatter-adds the two expert contributions per token.

Device layout (everything transposed so no on-device transposes are needed):
  xt  [D, C]   bf16   gathered tokens, transposed
  wg  [32, 128, 2048] bf16  = w_gate re-tiled so wg[f] is an SBUF tile
                              [k-part, f*128+j] per k-chunk (lhsT layout)
  wu  same as wg
  wd  [16, 128, 4096] bf16  = w_down re-tiled likewise (lhsT layout)
  sc  [128, C] f32    combine weight per token, broadcast over partitions
  yt  [D, C]   f32    output, transposed

Per core:  H^T[f,c] = sum_k wg[k,f] * xt[k,c]   (PSUM, 16 accumulating mms)
           P^T[f,c] = gelu_tanh(H^T) * U^T      (ACT + DVE, bf16 to SBUF)
           Y^T[d,c] = sum_f wd[f,d] * P^T[f,c]  (PSUM, 32 accumulating mms)
           out = Y^T * sc                        (DVE, fp32)
C (token capacity per expert) is processed in 2 column-groups so P^T stays
SBUF-resident (no DRAM spill); weights are re-read once per group.
"""
import os
import sys

sys.path.insert(0, "/opt/trn_rl_repo")

import ml_dtypes
import numpy as np

D = 2048
F = 4096
E = 8
KD = D // 128   # 16 k-chunks for gate/up contraction
FT = F // 128   # 32 f-tiles
DT = D // 128   # 16 d-tiles
SOFTCAP = 30.0

_compiled_cache = {}


def _col_subtiles(cg, max_n=512):
    """Split a column-group of width cg into matmul free-dim subtiles.

    bf16 moving operand supports N up to 1024 (PSUM tile spans 2 banks)."""
    subs = []
    off = 0
    while off < cg:
        s = min(max_n, cg - off)
        subs.append((off, s))
        off += s
    return subs


def _build(C):
    import concourse.bass as bass
    import concourse.tile as tile
    from concourse import bacc, mybir

    f32 = mybir.dt.float32
    bf16 = mybir.dt.bfloat16
    GELU = mybir.ActivationFunctionType.Gelu_apprx_tanh

    nc = bacc.Bacc("TRN2", target_bir_lowering=False, debug=False, num_devices=E, num_swdge_queues=4)
    xt = nc.dram_tensor("xt", [D, C], bf16, kind="ExternalInput").ap()
    wg = nc.dram_tensor("wg", [FT, 128, D], bf16, kind="ExternalInput").ap()
    wu = nc.dram_tensor("wu", [FT, 128, D], bf16, kind="ExternalInput").ap()
    wd = nc.dram_tensor("wd", [DT, 128, F], bf16, kind="ExternalInput").ap()
    sc = nc.dram_tensor("sc", [128, C], f32, kind="ExternalInput").ap()
    yt = nc.dram_tensor("yt", [D, C], f32, kind="ExternalOutput").ap()

    # two column-groups: first an exact multiple of 1024 (pure N=512 matmul
    # subtiles), remainder in the second
    g1 = min((C // 1024) * 1024, 1024)
    groups = [(0, g1), (g1, C - g1)] if 0 < g1 < C else [(0, C)]

    with tile.TileContext(nc) as tc:
        with (
            tc.tile_pool(name="xts", bufs=1) as xpool,
            tc.tile_pool(name="pt", bufs=1) as ptpool,
            tc.tile_pool(name="wgp", bufs=3) as wgpool,
            tc.tile_pool(name="wup", bufs=3) as wupool,
            tc.tile_pool(name="wdp", bufs=3) as wdpool,
            tc.tile_pool(name="gel", bufs=2) as gpool,
            tc.tile_pool(name="scp", bufs=1) as spool,
            tc.tile_pool(name="outp", bufs=4) as opool,
            tc.tile_pool(name="ps", bufs=8, space="PSUM") as psum,
        ):
            for g0, cg in groups:
                subs = _col_subtiles(cg)
                # quarter-tiles so early k-chunk matmuls wait on only 1/4 of
                # the token DMA; alternate the two HW-DGE queues
                xq = KD // 4
                xts_q = [
                    xpool.tile([128, xq, cg], bf16, name=f"xtsq{q}", tag=f"xtsq{q}")
                    for q in range(4)
                ]
                for k in range(KD):
                    dst = xts_q[k // xq][:, k % xq, :]
                    eng = nc.sync if k % 2 == 0 else nc.scalar
                    eng.dma_start(dst, xt[k * 128 : (k + 1) * 128, g0 : g0 + cg])
                pt = ptpool.tile([128, FT, cg], bf16, name="pt")

                for f in range(FT):
                    wgt = wgpool.tile([128, D], bf16, name="wgt")
                    nc.gpsimd.dma_start(wgt[:], wg[f])
                    wut = wupool.tile([128, D], bf16, name="wut")
                    nc.gpsimd.dma_start(wut[:], wu[f])

                    psH = [psum.tile([128, 512], f32, name="ps", tag="ps") for _ in subs]
                    for k in range(KD):
                        lhs = wgt[:, k * 128 : (k + 1) * 128]
                        for ci, (off, s) in enumerate(subs):
                            nc.tensor.matmul(
                                psH[ci][:, :s],
                                lhs,
                                xts_q[k // xq][:, k % xq, off : off + s],
                                start=(k == 0),
                                stop=(k == KD - 1),
                            )
                    gel = gpool.tile([128, cg], f32, name="gel")
                    for ci, (off, s) in enumerate(subs):
                        nc.scalar.activation(gel[:, off : off + s], psH[ci][:, :s], GELU)

                    psU = [psum.tile([128, 512], f32, name="ps", tag="ps") for _ in subs]
                    for k in range(KD):
                        lhs = wut[:, k * 128 : (k + 1) * 128]
                        for ci, (off, s) in enumerate(subs):
                            nc.tensor.matmul(
                                psU[ci][:, :s],
                                lhs,
                                xts_q[k // xq][:, k % xq, off : off + s],
                                start=(k == 0),
                                stop=(k == KD - 1),
                            )
                    for ci, (off, s) in enumerate(subs):
                        nc.vector.tensor_mul(
                            pt[:, f, off : off + s],
                            gel[:, off : off + s],
                            psU[ci][:, :s],
                        )

                sct = spool.tile([128, cg], f32, name="sct")
                nc.gpsimd.dma_start(sct[:], sc[:, g0 : g0 + cg])
                for d in range(DT):
                    wdt = wdpool.tile([128, F], bf16, name="wdt")
                    nc.gpsimd.dma_start(wdt[:], wd[d])
                    psY = [psum.tile([128, 512], f32, name="ps", tag="ps") for _ in subs]
                    for f in range(FT):
                        lhs = wdt[:, f * 128 : (f + 1) * 128]
                        for ci, (off, s) in enumerate(subs):
                            nc.tensor.matmul(
                                psY[ci][:, :s],
                                lhs,
                                pt[:, f, off : off + s],
                                start=(f == 0),
                                stop=(f == FT - 1),
                            )
                    for ci, (off, s) in enumerate(subs):
                        outt = opool.tile([128, 512], f32, name="outt", tag="outt")
                        nc.vector.tensor_mul(
                            outt[:, :s], psY[ci][:, :s], sct[:, off : off + s]
                        )
                        nc.sync.dma_start(
                            yt[d * 128 : (d + 1) * 128, g0 + off : g0 + off + s],
                            outt[:, :s],
                        )

    nc.compile()
    return nc


def _enable_ntff_tracing():
    """Register the axon NTFF profile hook (the image lacks antenv.axon_hooks,
    so trn_boot's registration silently degraded). Also stub the S3 artifact
    upload, which has no credentials in this container."""
    import types

    try:
        from antenv import axon_hooks  # noqa: F401
    except ImportError:
        import antenv

        mod = types.ModuleType("antenv.axon_hooks")
        holder = [None]
        mod.set_axon_ntff_profile_hook = lambda h: holder.__setitem__(0, h)
        mod.get_axon_ntff_profile_hook = lambda: holder[0]
        sys.modules["antenv.axon_hooks"] = mod
        antenv.axon_hooks = mod
        from trn_agent_boot.trn_boot import _ntff_profile_via_ctypes

        hook = _ntff_profile_via_ctypes("/opt/axon/libaxon_pjrt.so")
        if hook is not None:
            mod.set_axon_ntff_profile_hook(hook)
    from concourse import bass_utils as bu

    bu.upload_artifacts = lambda tmpdir: ""


def kernel(hidden_states, gate_w, w_gate, w_up, w_down, top_k):
    from concourse.bass_utils import run_bass_kernel_spmd

    x = np.ascontiguousarray(np.asarray(hidden_states, dtype=np.float32))
    gw = np.asarray(gate_w, dtype=np.float32)
    k = int(top_k)
    T = x.shape[0]

    # --- host router (matches reference: fp32 gate, tanh softcap, softmax) ---
    logits = (x @ gw).astype(np.float32)
    logits = np.tanh(logits / SOFTCAP) * SOFTCAP
    m = logits.max(axis=1, keepdims=True)
    ex = np.exp(logits - m)
    probs = (ex / ex.sum(axis=1, keepdims=True)).astype(np.float32)
    order = np.argsort(-probs, axis=1, kind="stable")[:, :k]

    tok_ids = []
    counts = np.zeros(E, np.int64)
    sel = np.zeros((T, E), bool)
    for j in range(k):
        sel[np.arange(T), order[:, j]] = True
    for e in range(E):
        ids = np.nonzero(sel[:, e])[0]
        tok_ids.append(ids)
        counts[e] = len(ids)
    C = max(256, int(np.ceil(counts.max() / 8) * 8))

    nc = _compiled_cache.get(C)
    if nc is None:
        nc = _build(C)
        _compiled_cache[C] = nc

    bf = ml_dtypes.bfloat16
    in_maps = []
    for e in range(E):
        ids = tok_ids[e]
        n = len(ids)
        xt = np.zeros((D, C), bf)
        xt[:, :n] = x[ids].T.astype(bf)
        s = np.zeros((C,), np.float32)
        s[:n] = probs[ids, e]
        sc = np.broadcast_to(s[None, :], (128, C)).copy()
        wg_r = np.ascontiguousarray(
            np.asarray(w_gate[e], np.float32)
            .reshape(KD, 128, FT, 128)
            .transpose(2, 1, 0, 3)
            .reshape(FT, 128, D)
        ).astype(bf)
        wu_r = np.ascontiguousarray(
            np.asarray(w_up[e], np.float32)
            .reshape(KD, 128, FT, 128)
            .transpose(2, 1, 0, 3)
            .reshape(FT, 128, D)
        ).astype(bf)
        wd_r = np.ascontiguousarray(
            np.asarray(w_down[e], np.float32)
            .reshape(FT, 128, DT, 128)
            .transpose(2, 1, 0, 3)
            .reshape(DT, 128, F)
        ).astype(bf)
        in_maps.append({"xt": xt, "wg": wg_r, "wu": wu_r, "wd": wd_r, "sc": sc})

    trace = bool(int(os.environ.get("MOE_TRACE", "0")))
    if trace:
        try:
            _enable_ntff_tracing()
        except Exception as exc:  # tracing is best-effort, never block results
            print(f"ntff tracing unavailable: {exc!r}")
            trace = False
    res = run_bass_kernel_spmd(nc, in_maps, list(range(E)), trace=trace)
    if trace:
        kernel.last_exec_time_ns = res.exec_time_ns
        kernel.last_trace = res.instructions_and_trace

    out = np.zeros((T, D), np.float32)
    for e in range(E):
        ids = tok_ids[e]
        n = len(ids)
        if n:
            out[ids] += res.results[e]["yt"][:, :n].T
    return out


# revision 21
# speedup vs baseline: 1.0066x; 1.0053x over previous
"""Grok1-style MoE (T=8192, D=2048, F=4096, E=8, top_k=2) on 8 trn2 NeuronCores.

Expert-parallel: one expert per core. The router (tiny fp32 GEMM, 0.03% of
FLOPs) runs on host to decide the token->expert sharding; each core runs the
dense fused FFN  scale * (gelu_tanh(x@wg) * (x@wu)) @ wd  over the ~2048
tokens routed to its expert (bf16 matmuls, fp32 PSUM accumulate); host
scatter-adds the two expert contributions per token.

Device layout (everything transposed so no on-device transposes are needed):
  xt  [D, C]   bf16   gathered tokens, transposed
  wg  [32, 128, 2048] bf16  = w_gate re-tiled so wg[f] is an SBUF tile
                              [k-part, f*128+j] per k-chunk (lhsT layout)
  wu  same as wg
  wd  [16, 128, 4096] bf16  = w_down re-tiled likewise (lhsT layout)
  sc  [128, C] f32    combine weight per token, broadcast over partitions
  yt  [D, C]   f32    output, transposed

Per core:  H^T[f,c] = sum_k wg[k,f] * xt[k,c]   (PSUM, 16 accumulating mms)
           P^T[f,c] = gelu_tanh(H^T) * U^T      (ACT + DVE, bf16 to SBUF)
           Y^T[d,c] = sum_f wd[f,d] * P^T[f,c]  (PSUM, 32 accumulating mms)
           out = Y^T * sc                        (DVE, fp32)
C (token capacity per expert) is processed in 2 column-groups so P^T stays
SBUF-resident (no DRAM spill); weights are re-read once per group.
"""
import os
import sys

sys.path.insert(0, "/opt/trn_rl_repo")

import ml_dtypes
import numpy as np

D = 2048
F = 4096
E = 8
KD = D // 128   # 16 k-chunks for gate/up contraction
FT = F // 128   # 32 f-tiles
DT = D // 128   # 16 d-tiles
SOFTCAP = 30.0

_compiled_cache = {}


def _col_subtiles(cg, max_n=512):
    """Split a column-group of width cg into matmul free-dim subtiles.

    bf16 moving operand supports N up to 1024 (PSUM tile spans 2 banks)."""
    subs = []
    off = 0
    while off < cg:
        s = min(max_n, cg - off)
        subs.append((off, s))
        off += s
    return subs


def _build(C):
    import concourse.bass as bass
    import concourse.tile as tile
    from concourse import bacc, mybir

    f32 = mybir.dt.float32
    bf16 = mybir.dt.bfloat16
    GELU = mybir.ActivationFunctionType.Gelu_apprx_tanh

    nc = bacc.Bacc("TRN2", target_bir_lowering=False, debug=False, num_devices=E, num_swdge_queues=4)
    xt = nc.dram_tensor("xt", [D, C], bf16, kind="ExternalInput").ap()
    wg = nc.dram_tensor("wg", [FT, 128, D], bf16, kind="ExternalInput").ap()
    wu = nc.dram_tensor("wu", [FT, 128, D], bf16, kind="ExternalInput").ap()
    wd = nc.dram_tensor("wd", [DT, 128, F], bf16, kind="ExternalInput").ap()
    sc = nc.dram_tensor("sc", [128, C], f32, kind="ExternalInput").ap()
    yt = nc.dram_tensor("yt", [D, C], f32, kind="ExternalOutput").ap()

    # column groups: small first group so PE starts after only ~2.1MB of
    # token DMA, 1024-wide middle groups (pure N=512 subtiles), remainder last
    groups = []
    off = 0
    first = min(512, C)
    groups.append((0, first))
    off = first
    while C - off > 1024:
        groups.append((off, 1024))
        off += 1024
    if C - off:
        groups.append((off, C - off))

    with tile.TileContext(nc) as tc:
        with (
            tc.tile_pool(name="xts", bufs=1) as xpool,
            tc.tile_pool(name="pt", bufs=1) as ptpool,
            tc.tile_pool(name="wgp", bufs=3) as wgpool,
            tc.tile_pool(name="wup", bufs=3) as wupool,
            tc.tile_pool(name="wdp", bufs=3) as wdpool,
            tc.tile_pool(name="gel", bufs=2) as gpool,
            tc.tile_pool(name="scp", bufs=1) as spool,
            tc.tile_pool(name="outp", bufs=4) as opool,
            tc.tile_pool(name="ps", bufs=8, space="PSUM") as psum,
        ):
            for g0, cg in groups:
                subs = _col_subtiles(cg)
                # quarter-tiles so early k-chunk matmuls wait on only 1/4 of
                # the token DMA; alternate the two HW-DGE queues
                xq = KD // 4
                xts_q = [
                    xpool.tile([128, xq, cg], bf16, name=f"xtsq{q}", tag=f"xtsq{q}")
                    for q in range(4)
                ]
                for k in range(KD):
                    dst = xts_q[k // xq][:, k % xq, :]
                    eng = nc.sync if k % 2 == 0 else nc.scalar
                    eng.dma_start(dst, xt[k * 128 : (k + 1) * 128, g0 : g0 + cg])
                pt = ptpool.tile([128, FT, cg], bf16, name="pt")

                for f in range(FT):
                    wgt = wgpool.tile([128, D], bf16, name="wgt")
                    nc.gpsimd.dma_start(wgt[:], wg[f])
                    wut = wupool.tile([128, D], bf16, name="wut")
                    nc.gpsimd.dma_start(wut[:], wu[f])

                    psH = [psum.tile([128, 512], f32, name="ps", tag="ps") for _ in subs]
                    for k in range(KD):
                        lhs = wgt[:, k * 128 : (k + 1) * 128]
                        for ci, (off, s) in enumerate(subs):
                            nc.tensor.matmul(
                                psH[ci][:, :s],
                                lhs,
                                xts_q[k // xq][:, k % xq, off : off + s],
                                start=(k == 0),
                                stop=(k == KD - 1),
                            )
                    gel = gpool.tile([128, cg], f32, name="gel")
                    for ci, (off, s) in enumerate(subs):
                        nc.scalar.activation(gel[:, off : off + s], psH[ci][:, :s], GELU)

                    psU = [psum.tile([128, 512], f32, name="ps", tag="ps") for _ in subs]
                    for k in range(KD):
                        lhs = wut[:, k * 128 : (k + 1) * 128]
                        for ci, (off, s) in enumerate(subs):
                            nc.tensor.matmul(
                                psU[ci][:, :s],
                                lhs,
                                xts_q[k // xq][:, k % xq, off : off + s],
                                start=(k == 0),
                                stop=(k == KD - 1),
                            )
                    for ci, (off, s) in enumerate(subs):
                        nc.vector.tensor_mul(
                            pt[:, f, off : off + s],
                            gel[:, off : off + s],
                            psU[ci][:, :s],
                        )

                sct = spool.tile([128, cg], f32, name="sct")
                nc.gpsimd.dma_start(sct[:], sc[:, g0 : g0 + cg])
                for d in range(DT):
                    wdt = wdpool.tile([128, F], bf16, name="wdt")
                    nc.gpsimd.dma_start(wdt[:], wd[d])
                    psY = [psum.tile([128, 512], f32, name="ps", tag="ps") for _ in subs]
                    for f in range(FT):
                        lhs = wdt[:, f * 128 : (f + 1) * 128]
                        for ci, (off, s) in enumerate(subs):
                            nc.tensor.matmul(
                                psY[ci][:, :s],
                                lhs,
                                pt[:, f, off : off + s],
                                start=(f == 0),
                                stop=(f == FT - 1),
                            )
                    for ci, (off, s) in enumerate(subs):
                        outt = opool.tile([128, 512], f32, name="outt", tag="outt")
                        nc.vector.tensor_mul(
                            outt[:, :s], psY[ci][:, :s], sct[:, off : off + s]
                        )
                        nc.sync.dma_start(
                            yt[d * 128 : (d + 1) * 128, g0 + off : g0 + off + s],
                            outt[:, :s],
                        )

    nc.compile()
    return nc


def _enable_ntff_tracing():
    """Register the axon NTFF profile hook (the image lacks antenv.axon_hooks,
    so trn_boot's registration silently degraded). Also stub the S3 artifact
    upload, which has no credentials in this container."""
    import types

    try:
        from antenv import axon_hooks  # noqa: F401
    except ImportError:
        import antenv

        mod = types.ModuleType("antenv.axon_hooks")
        holder = [None]
        mod.set_axon_ntff_profile_hook = lambda h: holder.__setitem__(0, h)
        mod.get_axon_ntff_profile_hook = lambda: holder[0]
        sys.modules["antenv.axon_hooks"] = mod
        antenv.axon_hooks = mod
        from trn_agent_boot.trn_boot import _ntff_profile_via_ctypes

        hook = _ntff_profile_via_ctypes("/opt/axon/libaxon_pjrt.so")
        if hook is not None:
            mod.set_axon_ntff_profile_hook(hook)
    from concourse import bass_utils as bu

    bu.upload_artifacts = lambda tmpdir: ""


def kernel(hidden_states, gate_w, w_gate, w_up, w_down, top_k):
    from concourse.bass_utils import run_bass_kernel_spmd

    x = np.ascontiguousarray(np.asarray(hidden_states, dtype=np.float32))
    gw = np.asarray(gate_w, dtype=np.float32)
    k = int(top_k)
    T = x.shape[0]

    # --- host router (matches reference: fp32 gate, tanh softcap, softmax) ---
    logits = (x @ gw).astype(np.float32)
    logits = np.tanh(logits / SOFTCAP) * SOFTCAP
    m = logits.max(axis=1, keepdims=True)
    ex = np.exp(logits - m)
    probs = (ex / ex.sum(axis=1, keepdims=True)).astype(np.float32)
    order = np.argsort(-probs, axis=1, kind="stable")[:, :k]

    tok_ids = []
    counts = np.zeros(E, np.int64)
    sel = np.zeros((T, E), bool)
    for j in range(k):
        sel[np.arange(T), order[:, j]] = True
    for e in range(E):
        ids = np.nonzero(sel[:, e])[0]
        tok_ids.append(ids)
        counts[e] = len(ids)
    C = max(256, int(np.ceil(counts.max() / 8) * 8))

    nc = _compiled_cache.get(C)
    if nc is None:
        nc = _build(C)
        _compiled_cache[C] = nc

    bf = ml_dtypes.bfloat16
    in_maps = []
    for e in range(E):
        ids = tok_ids[e]
        n = len(ids)
        xt = np.zeros((D, C), bf)
        xt[:, :n] = x[ids].T.astype(bf)
        s = np.zeros((C,), np.float32)
        s[:n] = probs[ids, e]
        sc = np.broadcast_to(s[None, :], (128, C)).copy()
        wg_r = np.ascontiguousarray(
            np.asarray(w_gate[e], np.float32)
            .reshape(KD, 128, FT, 128)
            .transpose(2, 1, 0, 3)
            .reshape(FT, 128, D)
        ).astype(bf)
        wu_r = np.ascontiguousarray(
            np.asarray(w_up[e], np.float32)
            .reshape(KD, 128, FT, 128)
            .transpose(2, 1, 0, 3)
            .reshape(FT, 128, D)
        ).astype(bf)
        wd_r = np.ascontiguousarray(
            np.asarray(w_down[e], np.float32)
            .reshape(FT, 128, DT, 128)
            .transpose(2, 1, 0, 3)
            .reshape(DT, 128, F)
        ).astype(bf)
        in_maps.append({"xt": xt, "wg": wg_r, "wu": wu_r, "wd": wd_r, "sc": sc})

    trace = bool(int(os.environ.get("MOE_TRACE", "0")))
    if trace:
        try:
            _enable_ntff_tracing()
        except Exception as exc:  # tracing is best-effort, never block results
            print(f"ntff tracing unavailable: {exc!r}")
            trace = False
    res = run_bass_kernel_spmd(nc, in_maps, list(range(E)), trace=trace)
    if trace:
        kernel.last_exec_time_ns = res.exec_time_ns
        kernel.last_trace = res.instructions_and_trace

    out = np.zeros((T, D), np.float32)
    for e in range(E):
        ids = tok_ids[e]
        n = len(ids)
        if n:
            out[ids] += res.results[e]["yt"][:, :n].T
    return out
